# revision 22
# baseline (speedup 1.0000x reference)
"""Trainium2 Bass kernel for the KNet-style recurrent chain (batch=1).

Strategy (memory-bound problem, ~353MB of fp32 weights):
  - The small GRU chain + small FCs (~35MB) are REPLICATED on all 8 cores.
  - FC2 (the big Kalman-gain MLP: W2a [46080,1152], W2b [576,46080]) is
    tensor-parallel: each core gets 5760 rows of W2a and the matching 5760
    columns of W2b, computes a partial y [576]; the host sums the 8 partials
    and adds b2b (the "all-reduce" done on host).
  - Every matvec y = W @ x runs on the TensorEngine in WEIGHT-MOVING form:
        out[1, N] (+)= x_chunk[K, 1].T @ W.T_chunk[K, N]
    i.e. the tiny activation chunk is the stationary operand (fast fp32
    load) and the pre-transposed weights stream as the moving operand
    (~430ns per [128, 512] fp32 block, ~611 GB/s — above the per-core HBM
    rate).  Keeping weights stationary instead costs ~350ns per [128,128]
    tile (fp32 weight load), 3x too slow.
  - Matvec outputs live in free-layout [1, M] (one partition); elementwise
    GRU math happens there; PE transpose-mode matmuls ([1,128] -> [128,1],
    ~330ns) rebuild the partition-layout [128, ceil(d/128)] tiles consumed
    as the next layer's stationary chunks.
  - PSUM accumulation: start=True clears has_written for the WHOLE target
    bank, so it is set only on the first matmul into each bank; later
    first-writes to an element overwrite because has_written=0.
"""

import sys

sys.path.insert(0, "/opt/trn_rl_repo")

import numpy as np

NCORES = 8
H = 576                      # hidden size of all three GRUs
D2_HID, D2_IN, D2_OUT = 46080, 1152, 576
MSH = D2_HID // NCORES       # 5760 rows of W2a per core
NM2 = MSH // 128             # 45 output chunks per core
STRIPE = 512                 # FC2a output stripe width
W2B_GRP = 3                  # FC2b K-blocks per DMA

F32 = np.float32


def _ncols(d):
    return (d + 127) // 128


def _nsplits(m):
    """split free dim at 512 boundaries (= PSUM bank boundaries)."""
    return [(n0, min(512, m - n0)) for n0 in range(0, m, 512)]


_CACHE = {}


class _Vec:
    """An activation vector in SBUF P-layout [128, ncols]."""

    def __init__(self, tile, d):
        self.tile = tile
        self.d = d

    def chunks(self):
        for c in range(_ncols(self.d)):
            sz = min(128, self.d - c * 128)
            yield self.tile[0:sz, c : c + 1], sz


def _build_program(dbg=False):
    import concourse.bass as bass  # noqa: F401
    from concourse import bacc, mybir
    import concourse.tile as tile

    f32 = mybir.dt.float32
    f32r = mybir.dt.float32r
    AF = mybir.ActivationFunctionType

    nc = bacc.Bacc(
        "TRN2", target_bir_lowering=False, debug=False, num_devices=NCORES
    )

    def din(name, shape, dt=f32):
        return nc.dram_tensor(name, list(shape), dt, kind="ExternalInput")

    # --- dram inputs: activation vectors ---
    d_x5 = din("x5", (24, 1), f32r)
    d_x6 = din("x6", (24, 1), f32r)
    d_obs = din("obs", (48, 1), f32r)
    d_hq = din("h_q", (128, 5), f32r)      # P-layout (matvec operand)
    d_hsig = din("h_sig", (128, 5), f32r)
    d_hs = din("h_s", (128, 5), f32r)
    d_hq_f = din("h_q_f", (1, H))    # free-layout (elementwise operand)
    d_hsig_f = din("h_sig_f", (1, H))
    d_hs_f = din("h_s_f", (1, H))

    # --- dram inputs: weights, host-stored as W.T [K, M] row-major ---
    wshapes = {
        "w5": (24, 480), "w6": (24, 480), "w7": (48, 960), "w1": (576, 576),
        "wrz_q": (1056, 1152), "win_q": (480, 576), "whn_q": (576, 576),
        "wrz_sig": (1632, 1152), "win_sig": (1056, 576), "whn_sig": (576, 576),
        "wrz_s": (2112, 1152), "win_s": (1536, 576), "whn_s": (576, 576),
        "w2a": (D2_IN, MSH), "w2b": (MSH, D2_OUT),
    }
    dw = {k: din(k, v, f32r) for k, v in wshapes.items()}

    # --- dram inputs: biases in free-layout [1, M] ---
    bshapes = {
        "b5": 480, "b6": 480, "b7": 960, "b1": H,
        "brz_q": 1152, "bin_q": H, "bhn_q": H,
        "brz_sig": 1152, "bin_sig": H, "bhn_sig": H,
        "brz_s": 1152, "bin_s": H, "bhn_s": H,
    }
    db = {k: din(k, (1, v)) for k, v in bshapes.items()}

    d_b2a = din("b2a", (1, MSH))
    d_y = nc.dram_tensor("y", [1, D2_OUT], f32, kind="ExternalOutput")

    dbg_outs = {}

    def _dbg(name, tile_ap, shape):
        if not dbg:
            return
        dt = nc.dram_tensor(f"dbg_{name}", list(shape), f32,
                            kind="ExternalOutput")
        nc.sync.dma_start(out=dt[:], in_=tile_ap.bitcast(f32))
        dbg_outs[name] = dt

    with tile.TileContext(nc) as tc:
        with (
            tc.tile_pool(name="const", bufs=1) as constp,
            tc.tile_pool(name="vecs", bufs=1) as vecp,
            tc.tile_pool(name="smallw", bufs=4) as swp,
            tc.tile_pool(name="bigw", bufs=3) as bigp,
            tc.tile_pool(name="w2bp", bufs=2) as w2bp,
            tc.tile_pool(name="ps", bufs=1, space="PSUM") as psp,
        ):
            def load_const(dram, shape, name, dt=f32):
                t = constp.tile(list(shape), dt, name=name, tag=name)
                nc.sync.dma_start(out=t, in_=dram[:])
                return t

            x5 = _Vec(load_const(d_x5, (24, 1), "t_x5", f32r), 24)
            x6 = _Vec(load_const(d_x6, (24, 1), "t_x6", f32r), 24)
            obs = _Vec(load_const(d_obs, (48, 1), "t_obs", f32r), 48)
            h_q = _Vec(load_const(d_hq, (128, 5), "t_hq", f32r), H)
            h_sig = _Vec(load_const(d_hsig, (128, 5), "t_hsig", f32r), H)
            h_s = _Vec(load_const(d_hs, (128, 5), "t_hs", f32r), H)
            hf = {
                "q": load_const(d_hq_f, (1, H), "t_hq_f"),
                "sig": load_const(d_hsig_f, (1, H), "t_hsig_f"),
                "s": load_const(d_hs_f, (1, H), "t_hs_f"),
            }
            bt = {
                k: load_const(db[k], (1, v), "t_" + k)
                for k, v in bshapes.items()
            }
            ident = constp.tile([1, 1], f32, name="ident", tag="ident")
            nc.vector.memset(ident, 1.0)

            def load_w_chunks(wname, segs, m_out):
                """DMA pre-transposed weights; yield (wt_ap, rhs, ksz)."""
                w = dw[wname]
                chunks = []
                ro = 0
                # cap tile size at ~14KB/partition
                grp = max(1, 14336 // (m_out * 4))
                for v in segs:
                    nb, tail = v.d // 128, v.d % 128
                    rhs_cols = list(v.chunks())
                    for g0 in range(0, nb, grp):
                        gn = min(grp, nb - g0)
                        wt = swp.tile([128, gn, m_out], f32r, tag="sw",
                                      name=f"w_{wname}_{ro}f{g0}", bufs=4)
                        nc.sync.dma_start(
                            out=wt,
                            in_=w[ro + g0 * 128 : ro + (g0 + gn) * 128,
                                  :].rearrange("(b p) m -> p b m", p=128),
                        )
                        for b in range(gn):
                            chunks.append(
                                (wt[:, b, :], rhs_cols[g0 + b][0], 128)
                            )
                    if tail:
                        wtt = swp.tile([tail, m_out], f32r, tag="sw",
                                       name=f"w_{wname}_{ro}t", bufs=4)
                        nc.sync.dma_start(
                            out=wtt, in_=w[ro + nb * 128 : ro + v.d, :]
                        )
                        chunks.append((wtt, rhs_cols[nb][0], tail))
                    ro += v.d
                return chunks

            def matvec_f(wname, segs, m_out, bias_tile, act, out_name,
                         psum_tag, psum_bufs, out_tag=None, out_bufs=2):
                """free-layout matvec: returns sbuf AP [1, m_out] of
                act(W @ concat(segs) + b)."""
                psum = psp.tile([1, max(m_out, 1152)], f32,
                                name=f"ps_{out_name}", tag=psum_tag,
                                bufs=psum_bufs)
                chunks = load_w_chunks(wname, segs, m_out)
                nch = len(chunks)
                for ci, (wt_ap, rhs, ksz) in enumerate(chunks):
                    for n0, nsz in _nsplits(m_out):
                        nc.tensor.matmul(
                            psum[0:1, n0 : n0 + nsz],
                            rhs,
                            wt_ap[0:ksz, n0 : n0 + nsz],
                            start=(ci == 0),
                            stop=(ci == nch - 1),
                            skip_group_check=True,
                        )
                out = vecp.tile([1, m_out], f32, name=out_name,
                                tag=out_tag or out_name,
                                bufs=out_bufs if out_tag else 1)
                nc.vector.tensor_add(out, psum[0:1, 0:m_out], bias_tile)
                if act is not None:
                    nc.scalar.activation(out, out, act)
                return out

            def to_play(free_ap, d, name):
                """transpose free-layout [1, d] -> P-layout [128, ncols]."""
                n_m = _ncols(d)
                ps_t = psp.tile([128, NM2], f32, name=f"pst_{name}",
                                tag="tp", bufs=1)
                for c in range(n_m):
                    csz = min(128, d - c * 128)
                    nc.tensor.matmul(
                        ps_t[0:csz, c : c + 1],
                        free_ap[0:1, c * 128 : c * 128 + csz],
                        ident,
                        is_transpose=True,
                        start=(c == 0),
                        stop=(c == n_m - 1),
                        skip_group_check=True,
                    )
                pl = vecp.tile([128, n_m], f32r, name=name, tag=name)
                nc.vector.tensor_copy(pl, ps_t[:, 0:n_m])
                return _Vec(pl, d)

            def gru(g, x_segs, h, out_name):
                rz = matvec_f(f"wrz_{g}", x_segs + [h], 2 * H,
                              bt[f"brz_{g}"], AF.Sigmoid, f"rz_{g}",
                              "mv1", 1, out_tag="rz_sb")
                gin = matvec_f(f"win_{g}", x_segs, H, bt[f"bin_{g}"], None,
                               f"gin_{g}", "mv1", 1, out_tag="gin_sb")
                ghn = matvec_f(f"whn_{g}", [h], H, bt[f"bhn_{g}"], None,
                               f"ghn_{g}", "mv1", 1, out_tag="ghn_sb")
                # n = tanh(gin + r * ghn);  h' = n + z * (h - n)
                t3 = vecp.tile([1, H], f32, name=f"t3_{g}", tag="t3",
                                bufs=1)
                nc.vector.tensor_mul(t3, rz[0:1, 0:H], ghn)
                nc.vector.tensor_add(t3, gin, t3)
                n_t = vecp.tile([1, H], f32, name=f"n_{g}", tag="n_t",
                                bufs=1)
                nc.scalar.activation(n_t, t3, AF.Tanh)
                t5 = vecp.tile([1, H], f32, name=f"t5_{g}", tag="t5",
                                bufs=1)
                nc.vector.tensor_sub(t5, hf[g], n_t)
                nc.vector.tensor_mul(t5, rz[0:1, H : 2 * H], t5)
                hn = vecp.tile([1, H], f32, name=out_name, tag="hn",
                                bufs=1)
                nc.vector.tensor_add(hn, n_t, t5)
                return hn

            # ---- the chain ----
            out5_f = matvec_f("w5", [x5], 480, bt["b5"], AF.Relu,
                              "out5_f", "mv1", 1, out_tag="vf")
            out5 = to_play(out5_f, 480, "out5")
            _dbg("out5", out5.tile, (128, 4))
            hQ_f = gru("q", [out5], h_q, "hQ_f")
            hQ = to_play(hQ_f, H, "hQ")
            _dbg("hQ", hQ.tile, (128, 5))
            out6_f = matvec_f("w6", [x6], 480, bt["b6"], AF.Relu,
                              "out6_f", "mv1", 1, out_tag="vf")
            out6 = to_play(out6_f, 480, "out6")
            _dbg("out6", out6.tile, (128, 4))
            hSig_f = gru("sig", [hQ, out6], h_sig, "hSig_f")
            hSig = to_play(hSig_f, H, "hSig")
            _dbg("hSig", hSig.tile, (128, 5))
            out1_f = matvec_f("w1", [hSig], H, bt["b1"], AF.Relu,
                              "out1_f", "mv1", 1, out_tag="vf")
            out1 = to_play(out1_f, H, "out1")
            _dbg("out1", out1.tile, (128, 5))
            out7_f = matvec_f("w7", [obs], 960, bt["b7"], AF.Relu,
                              "out7_f", "mv1", 1, out_tag="vf")
            out7 = to_play(out7_f, 960, "out7")
            _dbg("out7", out7.tile, (128, 8))
            hS_f = gru("s", [out1, out7], h_s, "hS_f")
            hS = to_play(hS_f, H, "hS")
            _dbg("hS", hS.tile, (128, 5))

            # ---- FC2a: h_fc = relu(W2a_shard @ [hSig, hS] + b2a_shard) ----
            # M-stripes of 512 outputs; per stripe the 10 K-chunks stream as
            # moving operand; output [1, nsz] accumulates in one PSUM bank.
            in2 = [hSig, hS]
            ps_hfc = psp.tile([128, NM2], f32, name="ps_hfc", tag="tp",
                              bufs=1)
            n_tp = 0
            for m0, nsz in _nsplits(MSH):
                psf = psp.tile([1, STRIPE], f32, name=f"ps_f{m0}",
                               tag="fca", bufs=2)
                b2s = vecp.tile([1, STRIPE], f32, name=f"b2s_{m0}",
                                tag="b2as", bufs=2)
                nc.sync.dma_start(out=b2s[0:1, 0:nsz],
                                  in_=d_b2a[0:1, m0 : m0 + nsz])
                hstr = vecp.tile([1, STRIPE], f32, name=f"hstr_{m0}",
                                 tag="hstr", bufs=2)
                # stripe weights: for each seg, 3D block + tail
                chunks = []
                ro = 0
                for v in in2:
                    nb, tail = v.d // 128, v.d % 128
                    rhs_cols = list(v.chunks())
                    wt = bigp.tile([128, nb, nsz], f32r, tag="w2a",
                                   name=f"w2a_{m0}_{ro}f", bufs=5)
                    nc.sync.dma_start(
                        out=wt,
                        in_=dw["w2a"][ro : ro + nb * 128,
                                      m0 : m0 + nsz].rearrange(
                            "(b p) m -> p b m", p=128
                        ),
                    )
                    for b in range(nb):
                        chunks.append((wt[:, b, :], rhs_cols[b][0], 128))
                    wtt = bigp.tile([tail, nsz], f32r, tag="w2at",
                                    name=f"w2a_{m0}_{ro}t", bufs=4)
                    nc.sync.dma_start(
                        out=wtt,
                        in_=dw["w2a"][ro + nb * 128 : ro + v.d,
                                      m0 : m0 + nsz],
                    )
                    chunks.append((wtt, rhs_cols[nb][0], tail))
                    ro += v.d
                nch = len(chunks)
                for ci, (wt_ap, rhs, ksz) in enumerate(chunks):
                    nc.tensor.matmul(
                        psf[0:1, 0:nsz],
                        rhs,
                        wt_ap[0:ksz, 0:nsz],
                        start=(ci == 0),
                        stop=(ci == nch - 1),
                        skip_group_check=True,
                    )
                # bias + relu into the free-layout accumulator
                nc.vector.tensor_add(
                    hstr[0:1, 0:nsz], psf[0:1, 0:nsz], b2s[0:1, 0:nsz]
                )
                nc.scalar.activation(
                    hstr[0:1, 0:nsz], hstr[0:1, 0:nsz], AF.Relu
                )
                # transpose this stripe into P-layout columns
                for c in range(nsz // 128):
                    col = m0 // 128 + c
                    nc.tensor.matmul(
                        ps_hfc[:, col : col + 1],
                        hstr[0:1, c * 128 : (c + 1) * 128],
                        ident,
                        is_transpose=True,
                        start=(n_tp == 0),
                        stop=(n_tp == NM2 - 1),
                        skip_group_check=True,
                    )
                    n_tp += 1
            h_fc = vecp.tile([128, NM2], f32r, name="h_fc", tag="h_fc")
            nc.vector.tensor_copy(h_fc, ps_hfc)
            _dbg("h_fc", h_fc, (128, NM2))

            # ---- FC2b: y_partial = W2b_shard @ h_fc  (out [1, 576]) ----
            ps512 = psp.tile([1, 512], f32, name="ps_y512", tag="y512",
                             bufs=1)
            ps64 = psp.tile([1, 64], f32, name="ps_y64", tag="y64", bufs=1)
            for g in range(NM2 // W2B_GRP):
                wt = w2bp.tile([128, W2B_GRP, D2_OUT], f32r, tag="w2b",
                               name=f"w2b_{g}", bufs=2)
                r0 = g * W2B_GRP * 128
                nc.sync.dma_start(
                    out=wt,
                    in_=dw["w2b"][r0 : r0 + W2B_GRP * 128, :].rearrange(
                        "(b p) m -> p b m", p=128
                    ),
                )
                for j in range(W2B_GRP):
                    kb = g * W2B_GRP + j
                    lhs = h_fc[:, kb : kb + 1]
                    nc.tensor.matmul(
                        ps512[0:1, :], lhs,
                        wt[:, j, 0:512],
                        start=(kb == 0), stop=(kb == NM2 - 1),
                        skip_group_check=True,
                    )
                    nc.tensor.matmul(
                        ps64[0:1, :], lhs,
                        wt[:, j, 512:576],
                        start=(kb == 0), stop=(kb == NM2 - 1),
                        skip_group_check=True,
                    )
            y_sb = constp.tile([1, D2_OUT], f32, name="y_sb", tag="y_sb")
            nc.vector.tensor_copy(y_sb[:, 0:512], ps512)
            nc.vector.tensor_copy(y_sb[:, 512:576], ps64)
            nc.sync.dma_start(out=d_y[:], in_=y_sb)

    nc.compile()
    return nc


def _get_program():
    if "nc" not in _CACHE:
        _CACHE["nc"] = _build_program()
    return _CACHE["nc"]


# ----------------------------------------------------------------------------
# host-side data prep
# ----------------------------------------------------------------------------


def _play(v, ncols):
    """length-d vector -> P-layout [128, ncols] (zero padded)."""
    v = np.asarray(v, F32).ravel()
    buf = np.zeros((ncols, 128), F32)
    buf.reshape(-1)[: v.size] = v
    return np.ascontiguousarray(buf.T)


def _prep_inputs(inputs):
    """Build the 8 per-core input maps from the full (unsharded) inputs."""
    g = {k: np.asarray(v, F32) for k, v in inputs.items()}

    common = {
        "x5": g["fw_evol_diff"].reshape(24, 1).copy(),
        "x6": g["fw_update_diff"].reshape(24, 1).copy(),
        "obs": np.concatenate(
            [g["obs_diff"], g["obs_innov_diff"]]
        ).reshape(48, 1).copy(),
        "h_q": _play(g["h_Q"], 5),
        "h_sig": _play(g["h_Sigma"], 5),
        "h_s": _play(g["h_S"], 5),
        "h_q_f": g["h_Q"].reshape(1, H).copy(),
        "h_sig_f": g["h_Sigma"].reshape(1, H).copy(),
        "h_s_f": g["h_S"].reshape(1, H).copy(),
        "w5": np.ascontiguousarray(g["W5"].T),
        "w6": np.ascontiguousarray(g["W6"].T),
        "w7": np.ascontiguousarray(g["W7"].T),
        "w1": np.ascontiguousarray(g["W1"].T),
        "b5": g["b5"].reshape(1, -1).copy(),
        "b6": g["b6"].reshape(1, -1).copy(),
        "b7": g["b7"].reshape(1, -1).copy(),
        "b1": g["b1"].reshape(1, -1).copy(),
    }
    for tag, suf in (("q", "Q"), ("sig", "Sig"), ("s", "S")):
        Wih, Whh = g[f"Wih_{suf}"], g[f"Whh_{suf}"]
        bih, bhh = g[f"bih_{suf}"], g[f"bhh_{suf}"]
        common[f"wrz_{tag}"] = np.ascontiguousarray(
            np.concatenate([Wih[0 : 2 * H], Whh[0 : 2 * H]], axis=1).T
        )
        common[f"win_{tag}"] = np.ascontiguousarray(Wih[2 * H :].T)
        common[f"whn_{tag}"] = np.ascontiguousarray(Whh[2 * H :].T)
        common[f"brz_{tag}"] = (bih[0 : 2 * H] + bhh[0 : 2 * H]).reshape(1, -1)
        common[f"bin_{tag}"] = bih[2 * H :].reshape(1, -1).copy()
        common[f"bhn_{tag}"] = bhh[2 * H :].reshape(1, -1).copy()

    in_maps = []
    for k in range(NCORES):
        m = dict(common)
        sl = slice(k * MSH, (k + 1) * MSH)
        m["w2a"] = np.ascontiguousarray(g["W2a"][sl, :].T)
        m["w2b"] = np.ascontiguousarray(g["W2b"][:, sl].T)
        m["b2a"] = g["b2a"][sl].reshape(1, -1).copy()
        in_maps.append(m)
    return in_maps


def run(trace=False, **inputs):
    from concourse.bass_utils import run_bass_kernel_spmd

    nc = _get_program()
    in_maps = _prep_inputs(inputs)
    res = run_bass_kernel_spmd(nc, in_maps, list(range(NCORES)), trace=trace)
    y = np.zeros(D2_OUT, np.float64)
    for r in res.results:
        y += r["y"].reshape(-1).astype(np.float64)
    out = (y.astype(F32) + np.asarray(inputs["b2b"], F32)).reshape(24, 24)
    return out, res


def kernel(**inputs):
    out, _ = run(trace=False, **inputs)
    return out
